# revision 10
# baseline (speedup 1.0000x reference)
"""Trainium2 Bass kernel for the hetero-GNN message-passing model.

Distribution (8 NeuronCores, SPMD; full inputs in, full outputs out):
  - Destination-range sharding: each core owns 12544 users / 2560 targets;
    each edge is routed to the core owning its destination node and kept
    destination-sorted (CSR-style graph preprocessing on the host -- index
    plumbing only; every FLOP on tensor values runs on device).
  - Forward aggregation (AP -> user/target segment mean): SWDGE dma_gather
    fetches premultiplied AP-table rows per edge; a one-hot selector (DVE
    is_equal vs iota) scatters them into 128-destination PSUM windows via
    TensorE matmul accumulation; degree normalization + bias + dst-term +
    leaky-relu fused into the window drain.
  - Reverse aggregation (user/target -> 512 APs): per-core partials via
    gathered destination rows + one-hot PSUM matmuls into 4 AP slabs,
    AllReduce'd across cores.
  - Node tables cross cores via AllGather collectives; edge head outputs are
    computed in the per-core sorted order and un-permuted on the host.

Design note: this hardware build has no usable atomic scatter-add
(dma_scatter_add drops concurrent duplicate-row updates -- measured), so all
segment sums are sorted-order PSUM matmul accumulations.
"""

import numpy as np
import ml_dtypes

N_AP, N_USER, N_TARGET = 512, 100000, 20000
E_SERVES, E_SENSES = 2000000, 500000
H = 64
NEG_SLOPE = 0.01
NC = 8

USH = 12544           # users per core  (98 windows x 128)
TSH = 2560            # targets per core (20 windows x 128)
NW_U, NW_T = USH // 128, TSH // 128
GQ = 25088            # user-table quarter rows (int16-safe)
U_TBL = GQ * 4 + 128  # 100480
T_TBL = TSH * NC + 128
A_TBL = 640
RC = 16               # tiles per rev/head gather chunk (2048 edges)

_CACHE = {}
_last_in_maps = None


def _round_up(x, m):
    return (x + m - 1) // m * m


def _wrap16(idx):
    n = idx.shape[0]
    a = np.ascontiguousarray(idx.astype(np.int16).reshape(n // 16, 16).T)
    return np.tile(a, (8, 1))


def _col128(arr, dtype):
    return np.ascontiguousarray(arr.reshape(-1, 128).T.astype(dtype))


# ------------------------------------------------------------------ host prep
def _prep_all(serves_src, serves_dst, senses_src, senses_dst):
    cores = []
    for c in range(NC):
        d = {}
        m1 = (serves_dst >= USH * c) & (serves_dst < USH * (c + 1))
        e1 = np.nonzero(m1)[0]
        src1, dst1 = serves_src[m1], serves_dst[m1]
        o = np.argsort(dst1, kind="stable")
        d["f1"] = (src1[o], (dst1 - USH * c)[o],
                   np.bincount((dst1 - USH * c) >> 7, minlength=NW_U))
        key = src1 >> 7
        o = np.argsort(key, kind="stable")
        d["r1"] = (src1[o], (dst1 - USH * c)[o], e1[o],
                   np.bincount(key[o], minlength=4))

        m2 = (senses_dst >= TSH * c) & (senses_dst < TSH * (c + 1))
        e2 = np.nonzero(m2)[0]
        src2, dst2 = senses_src[m2], senses_dst[m2]
        o = np.argsort(dst2, kind="stable")
        d["f2"] = (src2[o], (dst2 - TSH * c)[o],
                   np.bincount((dst2 - TSH * c) >> 7, minlength=NW_T))
        key = src2 >> 7
        o = np.argsort(key, kind="stable")
        d["r2"] = (src2[o], (dst2 - TSH * c)[o], e2[o],
                   np.bincount(key[o], minlength=4))
        cores.append(d)

    tpw_u = [int(_round_up(max(c["f1"][2][w] for c in cores), 128) // 128)
             for w in range(NW_U)]
    tpw_t = [int(_round_up(max(c["f2"][2][w] for c in cores), 128) // 128)
             for w in range(NW_T)]
    runs1 = [int(_round_up(max(c["r1"][3][k] for c in cores), 128) // 128)
             for k in range(4)]
    runs2 = [int(_round_up(max(c["r2"][3][k] for c in cores), 128) // 128)
             for k in range(4)]
    E1, E2 = sum(tpw_u) * 128, sum(tpw_t) * 128
    E1R, E2R = sum(runs1) * 128, sum(runs2) * 128
    meta = dict(tpw_u=tpw_u, tpw_t=tpw_t, runs1=runs1, runs2=runs2,
                E1=E1, E2=E2, E1R=E1R, E2R=E2R)

    deg_u = np.bincount(serves_dst, minlength=N_USER).astype(np.float32)
    deg_t = np.bincount(senses_dst, minlength=N_TARGET).astype(np.float32)
    deg_sa = np.bincount(serves_src, minlength=N_AP).astype(np.float32)
    deg_ta = np.bincount(senses_src, minlength=N_AP).astype(np.float32)
    cnt_a = np.zeros((128, 8), np.float32)
    for s in range(4):
        cnt_a[:, s] = deg_sa[s * 128:(s + 1) * 128]
        cnt_a[:, 4 + s] = deg_ta[s * 128:(s + 1) * 128]

    per_core = []
    for c in range(NC):
        io = {"cnt_a": cnt_a}
        for rel, tpw, E in (("f1", tpw_u, E1), ("f2", tpw_t, E2)):
            src_s, dst_s, counts = cores[c][rel]
            sa16 = np.full(E, 512, np.int64)
            suoff = np.zeros(E, np.float64)
            pos = ip = 0
            for w, tw in enumerate(tpw):
                n = int(counts[w])
                sa16[pos:pos + n] = src_s[ip:ip + n]
                suoff[pos:pos + n] = dst_s[ip:ip + n] & 127
                pos += tw * 128
                ip += n
            io[f"sa16_{rel}"] = _wrap16(sa16)
            io[f"suoff_{rel}"] = _col128(suoff, np.float16)
        for rel, runs, E in (("r1", runs1, E1R), ("r2", runs2, E2R)):
            src_o, dst_o, eglob, counts = cores[c][rel]
            dq16 = np.zeros(E, np.int64)
            saoff = np.full(E, 255.0, np.float64)
            perm = np.full(E, -1, np.int64)
            hia = np.full(E, 24, np.int64)
            haq = np.zeros(E, np.float64)
            hid = np.full(E, 196 if rel == "r1" else 80, np.int64)
            hdq = np.zeros(E, np.float64)
            pos = ip = 0
            for k, tw in enumerate(runs):
                n = int(counts[k])
                s_, d_ = src_o[ip:ip + n], dst_o[ip:ip + n]
                dq16[pos:pos + n] = d_
                saoff[pos:pos + n] = s_ & 127
                perm[pos:pos + n] = eglob[ip:ip + n]
                if rel == "r1":
                    hia[pos:pos + n] = 16 + (s_ >> 6)
                    haq[pos:pos + n] = s_ & 63
                    hid[pos:pos + n] = d_ >> 6
                    hdq[pos:pos + n] = d_ & 63
                else:
                    hia[pos:pos + n] = s_ >> 5
                    haq[pos:pos + n] = 2 * (s_ & 31)
                    hid[pos:pos + n] = d_ >> 5
                    hdq[pos:pos + n] = 2 * (d_ & 31)
                pos += tw * 128
                ip += n
            io[f"dq16_{rel}"] = _wrap16(dq16)
            io[f"saoff_{rel}"] = _col128(saoff, np.float16)
            io[f"hia16_{rel}"] = _wrap16(hia)
            io[f"haq_{rel}"] = _col128(haq, np.float16)
            io[f"hid16_{rel}"] = _wrap16(hid)
            io[f"hdq_{rel}"] = _col128(hdq, np.float16)
            io[f"perm_{rel}"] = perm
        du = np.zeros(USH, np.float32)
        hi = min(USH * (c + 1), N_USER)
        du[:hi - USH * c] = deg_u[USH * c:hi]
        io["cnt_u"] = _col128(du, np.float32)
        dt_ = np.zeros(TSH, np.float32)
        hi = min(TSH * (c + 1), N_TARGET)
        dt_[:hi - TSH * c] = deg_t[TSH * c:hi]
        io["cnt_t"] = _col128(dt_, np.float32)
        per_core.append(io)
    return meta, per_core


# ------------------------------------------------------------- device builder
def _build(meta):
    import concourse.bacc as bacc
    import concourse.mybir as mybir
    import concourse.tile as tile

    dt = mybir.dt
    AoT = mybir.AluOpType
    E1, E2, E1R, E2R = meta["E1"], meta["E2"], meta["E1R"], meta["E2R"]
    tpw_u, tpw_t = meta["tpw_u"], meta["tpw_t"]
    runs1, runs2 = meta["runs1"], meta["runs2"]
    TPW_MAX = max(max(tpw_u), max(tpw_t), RC)
    groups = [list(range(NC))]

    nc = bacc.Bacc("TRN2", target_bir_lowering=False, debug=False,
                   num_devices=NC)

    def din(name, shape, dtype):
        return nc.dram_tensor(name, shape, dtype, kind="ExternalInput").ap()

    def dout(name, shape, dtype):
        return nc.dram_tensor(name, shape, dtype, kind="ExternalOutput").ap()

    def dram(name, shape, dtype, shared=False):
        if shared:
            return nc.dram_tensor(name, shape, dtype, addr_space="Shared").ap()
        return nc.dram_tensor(name, shape, dtype).ap()

    xapT = din("xapT", [16, 512], dt.float32)
    xuT_in = din("xuT_in", [8, USH], dt.float32)
    xtT_in = din("xtT_in", [8, TSH], dt.float32)
    sa16_f1 = din("sa16_f1", [128, E1 // 16], dt.int16)
    suoff_f1 = din("suoff_f1", [128, E1 // 128], dt.float16)
    sa16_f2 = din("sa16_f2", [128, E2 // 16], dt.int16)
    suoff_f2 = din("suoff_f2", [128, E2 // 128], dt.float16)
    dq16_r1 = din("dq16_r1", [128, E1R // 16], dt.int16)
    saoff_r1 = din("saoff_r1", [128, E1R // 128], dt.float16)
    dq16_r2 = din("dq16_r2", [128, E2R // 16], dt.int16)
    saoff_r2 = din("saoff_r2", [128, E2R // 128], dt.float16)
    hia16_r1 = din("hia16_r1", [128, E1R // 16], dt.int16)
    haq_r1 = din("haq_r1", [128, E1R // 128], dt.float16)
    hid16_r1 = din("hid16_r1", [128, E1R // 16], dt.int16)
    hdq_r1 = din("hdq_r1", [128, E1R // 128], dt.float16)
    hia16_r2 = din("hia16_r2", [128, E2R // 16], dt.int16)
    haq_r2 = din("haq_r2", [128, E2R // 128], dt.float16)
    hid16_r2 = din("hid16_r2", [128, E2R // 16], dt.int16)
    hdq_r2 = din("hdq_r2", [128, E2R // 128], dt.float16)
    cnt_u = din("cnt_u", [128, NW_U], dt.float32)
    cnt_t = din("cnt_t", [128, NW_T], dt.float32)
    cnt_a = din("cnt_a", [128, 8], dt.float32)
    pwa = din("pwa", [16, H], dt.float32)
    pwu = din("pwu", [8, H], dt.float32)
    pwt = din("pwt", [8, H], dt.float32)
    pb = din("pb", [128, 3 * H], dt.float32)     # host-replicated biases
    WL = din("WL", [64, 8 * H], dt.float32)
    WR = din("WR", [64, 8 * H], dt.float32)
    BL = din("BL", [128, 8 * H], dt.float32)
    WHEAD = din("WHEAD", [64, 8], dt.float32)
    BHEAD = din("BHEAD", [128, 8], dt.float32)

    o_tau = dout("o_tau", [128, 4], dt.float32)
    o_s = dout("o_s", [128, NW_T], dt.float32)
    o_xlog = dout("o_xlog", [128, E1R // 128], dt.float32)
    o_ytx = dout("o_ytx", [128, E2R // 128], dt.float32)
    o_yrx = dout("o_yrx", [128, E2R // 128], dt.float32)

    tblA = dram("tblA", [A_TBL, 64], dt.float32)
    tblU = dram("tblU", [USH + 128, 64], dt.float32)
    tblT = dram("tblT", [TSH + 128, 64], dt.float32)
    rev_in = dram("rev_in", [8, 128, H], dt.float32)
    rev_out = dram("rev_out", [8, 128, H], dt.float32, shared=True)
    tblHA = dram("tblHA", [32, 64], dt.float32)
    tblHU = dram("tblHU", [200, 64], dt.float32)
    tblHT = dram("tblHT", [84, 64], dt.float32)

    with tile.TileContext(nc) as tc:
        with (tc.tile_pool(name="c", bufs=1) as cp,
              tc.tile_pool(name="w", bufs=2) as wp,
              tc.tile_pool(name="g", bufs=2) as gp,
              tc.tile_pool(name="p", bufs=4, space="PSUM") as pp,
              tc.tile_pool(name="pt", bufs=2, space="PSUM") as ptp):

            # ---------------- constants ----------------
            iota_i = cp.tile([128, 128], dt.int16, tag="iota_i")
            nc.gpsimd.iota(iota_i[:], [[1, 128]], channel_multiplier=0)
            iota_f = cp.tile([128, 128], dt.float16, tag="iota_f")
            nc.vector.tensor_copy(iota_f[:], iota_i[:])
            iota_pi = cp.tile([128, 1], dt.int16, tag="iota_pi")
            nc.gpsimd.iota(iota_pi[:], [[0, 1]], channel_multiplier=1)
            iota_pf = cp.tile([128, 1], dt.float16, tag="iota_pf")
            nc.vector.tensor_copy(iota_pf[:], iota_pi[:])
            ident = cp.tile([128, 128], dt.float32, tag="ident")
            nc.vector.tensor_tensor(out=ident[:],
                                    in0=iota_pf[:].to_broadcast([128, 128]),
                                    in1=iota_f[:], op=AoT.is_equal)

            WLs = cp.tile([64, 8 * H], dt.float32, tag="WLs")
            nc.sync.dma_start(out=WLs[:], in_=WL[:])
            WRs = cp.tile([64, 8 * H], dt.float32, tag="WRs")
            nc.sync.dma_start(out=WRs[:], in_=WR[:])
            WRb = cp.tile([64, 8 * H], dt.bfloat16, tag="WRb")
            nc.vector.tensor_copy(WRb[:], WRs[:])
            BLs = cp.tile([128, 8 * H], dt.float32, tag="BLs")
            nc.sync.dma_start(out=BLs[:], in_=BL[:])
            WHs = cp.tile([64, 8], dt.float32, tag="WHs")
            nc.sync.dma_start(out=WHs[:], in_=WHEAD[:])
            WHb = cp.tile([64, 8], dt.bfloat16, tag="WHb")
            nc.vector.tensor_copy(WHb[:], WHs[:])
            BHs = cp.tile([128, 8], dt.float32, tag="BHs")
            nc.sync.dma_start(out=BHs[:], in_=BHEAD[:])

            def load_inv(ap, n, tag):
                t = cp.tile([128, n], dt.float32, tag=tag)
                nc.sync.dma_start(out=t[:], in_=ap[:])
                nc.vector.tensor_scalar_max(t[:], t[:], 1.0)
                nc.vector.reciprocal(t[:], t[:])
                return t
            inv_u = load_inv(cnt_u, NW_U, "inv_u")
            inv_t = load_inv(cnt_t, NW_T, "inv_t")
            inv_a = load_inv(cnt_a, 8, "inv_a")

            xaT = cp.tile([64, 512], dt.float32, tag="xaT")
            xuT = cp.tile([64, USH], dt.bfloat16, tag="xuT")
            xtT = cp.tile([64, TSH], dt.bfloat16, tag="xtT")
            xuT2 = cp.tile([64, USH], dt.bfloat16, tag="xuT2")
            xtT2 = cp.tile([64, TSH], dt.bfloat16, tag="xtT2")

            def lrelu(ap, shape):
                t = wp.tile(shape, dt.float32, tag="lr")
                nc.vector.tensor_scalar_mul(t[:], ap, NEG_SLOPE)
                nc.vector.tensor_tensor(out=ap, in0=ap, in1=t[:], op=AoT.max)

            def transpose_to(destT_slice, ob_ap, out_dtype_bf):
                """destT_slice [64, 128] <- ob [128, 64]^T via PE."""
                psT = ptp.tile([64, 128], dt.float32, tag="pt")
                nc.tensor.transpose(out=psT[:], in_=ob_ap, identity=ident[:])
                nc.vector.tensor_copy(destT_slice, psT[:])

            # ---------------- projections ----------------
            xapT_s = wp.tile([16, 512], dt.float32, tag="xapT_s")
            nc.sync.dma_start(out=xapT_s[:], in_=xapT[:])
            pwa_s = cp.tile([16, H], dt.float32, tag="pwa_s")
            nc.sync.dma_start(out=pwa_s[:], in_=pwa[:])
            pb_s = cp.tile([128, 3 * H], dt.float32, tag="pb_s")
            nc.sync.dma_start(out=pb_s[:], in_=pb[:])
            for s in range(4):
                ps = pp.tile([128, H], dt.float32, tag="p")
                nc.tensor.matmul(ps[:], lhsT=xapT_s[:, s * 128:(s + 1) * 128],
                                 rhs=pwa_s[:], start=True, stop=True)
                ob = wp.tile([128, H], dt.float32, tag="ob")
                nc.vector.scalar_tensor_tensor(
                    out=ob[:], in0=ps[:], scalar=1.0, in1=pb_s[:, 0:H],
                    op0=AoT.mult, op1=AoT.add)
                transpose_to(xaT[:, s * 128:(s + 1) * 128], ob[:], False)

            def proj_nodes(xT_in, pw_ap, bias_off, destT, nwin, tbl_loc, ncols):
                pw_s = wp.tile([8, H], dt.float32, tag="pw_s")
                nc.sync.dma_start(out=pw_s[:], in_=pw_ap[:])
                for w in range(nwin):
                    xTw = wp.tile([8, 128], dt.float32, tag="xTw")
                    nc.sync.dma_start(out=xTw[:], in_=xT_in[:, w * 128:(w + 1) * 128])
                    ps = pp.tile([128, H], dt.float32, tag="p")
                    nc.tensor.matmul(ps[:], lhsT=xTw[:],
                                     rhs=pw_s[:], start=True, stop=True)
                    ob = wp.tile([128, H], dt.float32, tag="ob")
                    nc.vector.scalar_tensor_tensor(
                        out=ob[:], in0=ps[:], scalar=1.0,
                        in1=pb_s[:, bias_off:bias_off + H],
                        op0=AoT.mult, op1=AoT.add)
                    transpose_to(destT[:, w * 128:(w + 1) * 128], ob[:], True)
                    nc.sync.dma_start(out=tbl_loc[w * 128:(w + 1) * 128, :],
                                      in_=ob[:])

            proj_nodes(xuT_in, pwu, H, xuT, NW_U, tblU, USH)
            proj_nodes(xtT_in, pwt, 2 * H, xtT, NW_T, tblT, TSH)

            def build_tblA(wl_col):
                for s in range(4):
                    ps = pp.tile([128, H], dt.float32, tag="p")
                    nc.tensor.matmul(ps[:], lhsT=xaT[:, s * 128:(s + 1) * 128],
                                     rhs=WLs[:, wl_col:wl_col + H],
                                     start=True, stop=True)
                    pk = wp.tile([128, H], dt.float32, tag="pk")
                    nc.vector.tensor_copy(pk[:], ps[:])
                    nc.sync.dma_start(out=tblA[s * 128:(s + 1) * 128, :],
                                      in_=pk[:])
                zr = wp.tile([128, H], dt.float32, tag="pk")
                nc.vector.memset(zr[:], 0.0)
                nc.sync.dma_start(out=tblA[512:640, :], in_=zr[:])

            # ------------- forward pass -------------
            def fwd(sa16, suoff, tpw, inv_dst, col, srcT, destT, tbl_loc, layer):
                t0 = 0
                for w, ntile in enumerate(tpw):
                    psd = pp.tile([128, H], dt.float32, tag="p")
                    nc.tensor.matmul(psd[:], lhsT=srcT[:, w * 128:(w + 1) * 128],
                                     rhs=WRb[:, col:col + H], start=True, stop=True)
                    ob = wp.tile([128, H], dt.float32, tag="ob")
                    if ntile > 0:
                        ix = gp.tile([128, TPW_MAX * 8], dt.int16, tag="ix")
                        nc.sync.dma_start(out=ix[:, :ntile * 8],
                                          in_=sa16[:, t0 * 8:(t0 + ntile) * 8])
                        g = gp.tile([128, TPW_MAX, 64], dt.float32, tag="gg")
                        nc.gpsimd.dma_gather(
                            g[:, :ntile, :], tblA[:], ix[:, :ntile * 8],
                            ntile * 128, ntile * 128, 64, single_packet=False)
                        so = gp.tile([128, TPW_MAX], dt.float16, tag="so")
                        nc.sync.dma_start(out=so[:, :ntile],
                                          in_=suoff[:, t0:t0 + ntile])
                        oh = gp.tile([128, TPW_MAX, 128], dt.float32, tag="oh")
                        nc.vector.tensor_tensor(
                            out=oh[:, :ntile, :],
                            in0=so[:, :ntile, None].to_broadcast([128, ntile, 128]),
                            in1=iota_f[:, None, :].to_broadcast([128, ntile, 128]),
                            op=AoT.is_equal)
                        psa = pp.tile([128, H], dt.float32, tag="p")
                        for t in range(ntile):
                            nc.tensor.matmul(psa[:], lhsT=oh[:, t, :],
                                             rhs=g[:, t, :],
                                             start=(t == 0), stop=(t == ntile - 1))
                        nc.vector.tensor_scalar(
                            out=ob[:], in0=psa[:], scalar1=inv_dst[:, w:w + 1],
                            scalar2=None, op0=AoT.mult)
                        nc.vector.tensor_tensor(out=ob[:], in0=ob[:], in1=psd[:],
                                                op=AoT.add)
                    else:
                        nc.vector.tensor_copy(ob[:], psd[:])
                    nc.vector.tensor_tensor(out=ob[:], in0=ob[:],
                                            in1=BLs[:, col:col + H], op=AoT.add)
                    lrelu(ob[:], [128, H])
                    transpose_to(destT[:, w * 128:(w + 1) * 128], ob[:], True)
                    if layer == 1:
                        nc.sync.dma_start(out=tbl_loc[w * 128:(w + 1) * 128, :],
                                          in_=ob[:])
                    t0 += ntile

            # ------------- reverse pass -------------
            def rev(dq16, saoff, runs, tbl, revacc, acc_off):
                t0 = 0
                for s, ntile in enumerate(runs):
                    if ntile == 0:
                        continue
                    psr = pp.tile([128, H], dt.float32, tag="p")
                    nchunks = (ntile + RC - 1) // RC
                    for ci in range(nchunks):
                        c0 = ci * RC
                        nt = min(RC, ntile - c0)
                        ix = gp.tile([128, TPW_MAX * 8], dt.int16, tag="ix")
                        nc.sync.dma_start(
                            out=ix[:, :nt * 8],
                            in_=dq16[:, (t0 + c0) * 8:(t0 + c0 + nt) * 8])
                        g = gp.tile([128, TPW_MAX, 64], dt.float32, tag="gg")
                        nc.gpsimd.dma_gather(
                            g[:, :nt, :], tbl[:], ix[:, :nt * 8],
                            nt * 128, nt * 128, 64, single_packet=False)
                        so = gp.tile([128, TPW_MAX], dt.float16, tag="so")
                        nc.sync.dma_start(out=so[:, :nt],
                                          in_=saoff[:, t0 + c0:t0 + c0 + nt])
                        oh = gp.tile([128, TPW_MAX, 128], dt.float32, tag="oh")
                        nc.vector.tensor_tensor(
                            out=oh[:, :nt, :],
                            in0=so[:, :nt, None].to_broadcast([128, nt, 128]),
                            in1=iota_f[:, None, :].to_broadcast([128, nt, 128]),
                            op=AoT.is_equal)
                        for t in range(nt):
                            nc.tensor.matmul(
                                psr[:], lhsT=oh[:, t, :], rhs=g[:, t, :],
                                start=(ci == 0 and t == 0),
                                stop=(ci == nchunks - 1 and t == nt - 1))
                    nc.vector.tensor_tensor(
                        out=revacc[:, acc_off + s, :],
                        in0=revacc[:, acc_off + s, :], in1=psr[:], op=AoT.add)
                    t0 += ntile

            # ------------- AP update -------------
            def ap_update(revacc, layer):
                nc.sync.dma_start(out=rev_in[:], in_=revacc[:])
                nc.gpsimd.collective_compute(
                    "AllReduce", AoT.add, replica_groups=groups,
                    ins=[rev_in[:]], outs=[rev_out[:]])
                ra = wp.tile([128, 8, H], dt.float32, tag="ra")
                nc.sync.dma_start(out=ra[:], in_=rev_out[:])
                base = (layer - 1) * 4 * H
                wrsum = wp.tile([64, H], dt.float32, tag="wrsum")
                nc.vector.tensor_tensor(out=wrsum[:],
                                        in0=WRs[:, base + 2 * H:base + 3 * H],
                                        in1=WRs[:, base + 3 * H:base + 4 * H],
                                        op=AoT.add)
                for s in range(4):
                    ms = wp.tile([128, H], dt.float32, tag="ms")
                    nc.vector.tensor_scalar(
                        out=ms[:], in0=ra[:, s, :], scalar1=inv_a[:, s:s + 1],
                        scalar2=None, op0=AoT.mult)
                    msT = ptp.tile([64, 128], dt.float32, tag="pt")
                    nc.tensor.transpose(out=msT[:], in_=ms[:],
                                        identity=ident[:])
                    msTs = wp.tile([64, 128], dt.float32, tag="msTs")
                    nc.vector.tensor_copy(msTs[:], msT[:])
                    mt = wp.tile([128, H], dt.float32, tag="mt")
                    nc.vector.tensor_scalar(
                        out=mt[:], in0=ra[:, 4 + s, :],
                        scalar1=inv_a[:, 4 + s:5 + s], scalar2=None, op0=AoT.mult)
                    mtT = ptp.tile([64, 128], dt.float32, tag="pt")
                    nc.tensor.transpose(out=mtT[:], in_=mt[:],
                                        identity=ident[:])
                    mtTs = wp.tile([64, 128], dt.float32, tag="mtTs")
                    nc.vector.tensor_copy(mtTs[:], mtT[:])
                    ps = pp.tile([128, H], dt.float32, tag="p")
                    nc.tensor.matmul(ps[:], lhsT=msTs[:],
                                     rhs=WLs[:, base + 2 * H:base + 3 * H],
                                     start=True, stop=False)
                    nc.tensor.matmul(ps[:], lhsT=mtTs[:],
                                     rhs=WLs[:, base + 3 * H:base + 4 * H],
                                     start=False, stop=False)
                    nc.tensor.matmul(ps[:], lhsT=xaT[:, s * 128:(s + 1) * 128],
                                     rhs=wrsum[:], start=False, stop=True)
                    ob = wp.tile([128, H], dt.float32, tag="ob")
                    nc.vector.tensor_tensor(out=ob[:], in0=ps[:],
                                            in1=BLs[:, base + 2 * H:base + 3 * H],
                                            op=AoT.add)
                    nc.vector.tensor_tensor(out=ob[:], in0=ob[:],
                                            in1=BLs[:, base + 3 * H:base + 4 * H],
                                            op=AoT.add)
                    lrelu(ob[:], [128, H])
                    # xaT is read by matmuls above for all slabs; stage new
                    # values and write back after? (slab s update only reads
                    # slab s of xaT -> safe in-place per slab)
                    transpose_to(xaT[:, s * 128:(s + 1) * 128], ob[:], False)

            # =================== layer 1 ===================
            revacc = cp.tile([128, 8, H], dt.float32, tag="revacc")
            nc.vector.memset(revacc[:], 0.0)
            rev(dq16_r1, saoff_r1, runs1, tblU, revacc, 0)
            rev(dq16_r2, saoff_r2, runs2, tblT, revacc, 4)
            build_tblA(0 * H)
            fwd(sa16_f1, suoff_f1, tpw_u, inv_u, 0 * H, xuT, xuT2, tblU, 1)
            build_tblA(1 * H)
            fwd(sa16_f2, suoff_f2, tpw_t, inv_t, 1 * H, xtT, xtT2, tblT, 1)
            ap_update(revacc, 1)

            # =================== layer 2 ===================
            revacc2 = cp.tile([128, 8, H], dt.float32, tag="revacc2")
            nc.vector.memset(revacc2[:], 0.0)
            rev(dq16_r1, saoff_r1, runs1, tblU, revacc2, 0)
            rev(dq16_r2, saoff_r2, runs2, tblT, revacc2, 4)
            build_tblA(4 * H)
            fwd(sa16_f1, suoff_f1, tpw_u, inv_u, 4 * H, xuT2, xuT, None, 2)
            build_tblA(5 * H)
            fwd(sa16_f2, suoff_f2, tpw_t, inv_t, 5 * H, xtT2, xtT, None, 2)
            ap_update(revacc2, 2)

            # =================== head node-scalars ===================
            taus = wp.tile([128, 4], dt.float32, tag="taus")
            for s in range(4):
                ps = pp.tile([128, H], dt.float32, tag="p")
                nc.tensor.matmul(ps[:, :8], lhsT=xaT[:, s * 128:(s + 1) * 128],
                                 rhs=WHs[:], start=True, stop=True)
                hb = wp.tile([128, 1], dt.float32, tag="hb")
                nc.vector.tensor_tensor(out=hb[:], in0=ps[:, 0:1],
                                        in1=BHs[:, 0:1], op=AoT.add)
                nc.vector.tensor_copy(taus[:, s:s + 1], hb[:, 0:1])
                # pa (no bias; bx folded into pu) -> tblHA rows 16+2s..17+2s
                pa2 = wp.tile([128, 1], dt.float32, tag="pa2")
                nc.vector.tensor_copy(pa2[:], ps[:, 2:3])
                nc.sync.dma_start(out=tblHA[16 + 2 * s:18 + 2 * s, :],
                                  in_=pa2[:])
                # (qa, ra) interleaved, unbiased -> tblHA rows 4s..4s+3
                hb2 = wp.tile([128, 2], dt.float32, tag="hb2")
                nc.vector.tensor_copy(hb2[:, 0:1], ps[:, 4:5])
                nc.vector.tensor_copy(hb2[:, 1:2], ps[:, 6:7])
                nc.sync.dma_start(out=tblHA[4 * s:4 * s + 4, :], in_=hb2[:])
            nc.sync.dma_start(out=o_tau[:], in_=taus[:])
            zr8 = wp.tile([128, 2], dt.float32, tag="zr8")
            nc.vector.memset(zr8[:], 0.0)
            nc.sync.dma_start(out=tblHA[24:28, :], in_=zr8[:])
            nc.sync.dma_start(out=tblHA[28:32, :], in_=zr8[:])
            nc.sync.dma_start(out=tblHU[196:200, :], in_=zr8[:])
            nc.sync.dma_start(out=tblHT[80:84, :], in_=zr8[:])

            for w in range(NW_U):
                ps = pp.tile([128, H], dt.float32, tag="p")
                nc.tensor.matmul(ps[:, :8], lhsT=xuT[:, w * 128:(w + 1) * 128],
                                 rhs=WHb[:], start=True, stop=True)
                pu = wp.tile([128, 1], dt.float32, tag="pu")
                nc.vector.tensor_tensor(out=pu[:], in0=ps[:, 3:4],
                                        in1=BHs[:, 2:3], op=AoT.add)
                nc.sync.dma_start(out=tblHU[2 * w:2 * w + 2, :], in_=pu[:])
            souts = wp.tile([128, NW_T], dt.float32, tag="souts")
            for w in range(NW_T):
                ps = pp.tile([128, H], dt.float32, tag="p")
                nc.tensor.matmul(ps[:, :8], lhsT=xtT[:, w * 128:(w + 1) * 128],
                                 rhs=WHb[:], start=True, stop=True)
                nc.vector.tensor_tensor(out=souts[:, w:w + 1], in0=ps[:, 1:2],
                                        in1=BHs[:, 1:2], op=AoT.add)
                qts = wp.tile([128, 2], dt.float32, tag="qts")
                nc.vector.tensor_tensor(out=qts[:, 0:1], in0=ps[:, 5:6],
                                        in1=BHs[:, 3:4], op=AoT.add)
                nc.vector.tensor_tensor(out=qts[:, 1:2], in0=ps[:, 7:8],
                                        in1=BHs[:, 4:5], op=AoT.add)
                nc.sync.dma_start(out=tblHT[4 * w:4 * w + 4, :], in_=qts[:])
            nc.sync.dma_start(out=o_s[:], in_=souts[:])

            # =================== per-edge heads ===================
            def heads(hia16, haq, hid16, hdq, runs, outs_spec):
                """outs_spec: list of (out_ap, a_shift, d_shift)."""
                t0 = 0
                for k, ntile in enumerate(runs):
                    if ntile == 0:
                        continue
                    for c0 in range(0, ntile, RC):
                        nt = min(RC, ntile - c0)
                        sl16 = slice((t0 + c0) * 8, (t0 + c0 + nt) * 8)
                        sl = slice(t0 + c0, t0 + c0 + nt)
                        ixa = gp.tile([128, TPW_MAX * 8], dt.int16, tag="ix")
                        nc.sync.dma_start(out=ixa[:, :nt * 8], in_=hia16[:, sl16])
                        ga = gp.tile([128, RC, 64], dt.float32, tag="hga")
                        nc.gpsimd.dma_gather(
                            ga[:, :nt, :], tblHA[:], ixa[:, :nt * 8],
                            nt * 128, nt * 128, 64, single_packet=False)
                        ixd = gp.tile([128, TPW_MAX * 8], dt.int16, tag="ix2")
                        nc.sync.dma_start(out=ixd[:, :nt * 8], in_=hid16[:, sl16])
                        gd = gp.tile([128, RC, 64], dt.float32, tag="hgd")
                        nc.gpsimd.dma_gather(
                            gd[:, :nt, :], tblHU[:] if runs is runs1 else tblHT[:],
                            ixd[:, :nt * 8],
                            nt * 128, nt * 128, 64, single_packet=False)
                        qa_ = gp.tile([128, TPW_MAX], dt.float16, tag="so")
                        nc.sync.dma_start(out=qa_[:, :nt], in_=haq[:, sl])
                        qd_ = gp.tile([128, TPW_MAX], dt.float16, tag="so2")
                        nc.sync.dma_start(out=qd_[:, :nt], in_=hdq[:, sl])
                        ohA = gp.tile([128, RC, 64], dt.float32, tag="ohA")
                        nc.vector.tensor_tensor(
                            out=ohA[:, :nt, :],
                            in0=qa_[:, :nt, None].to_broadcast([128, nt, 64]),
                            in1=iota_f[:, None, :64].to_broadcast([128, nt, 64]),
                            op=AoT.is_equal)
                        ohD = gp.tile([128, RC, 64], dt.float32, tag="ohD")
                        nc.vector.tensor_tensor(
                            out=ohD[:, :nt, :],
                            in0=qd_[:, :nt, None].to_broadcast([128, nt, 64]),
                            in1=iota_f[:, None, :64].to_broadcast([128, nt, 64]),
                            op=AoT.is_equal)
                        tmp = gp.tile([128, RC, 64], dt.float32, tag="tmp")
                        for oap, ash, dsh in outs_spec:
                            va = wp.tile([128, RC], dt.float32, tag="va")
                            nc.vector.tensor_tensor(
                                out=tmp[:, :nt, :64 - ash],
                                in0=ga[:, :nt, ash:], in1=ohA[:, :nt, :64 - ash],
                                op=AoT.mult)
                            if ash:
                                nc.vector.memset(tmp[:, :nt, 64 - ash:], 0.0)
                            nc.vector.tensor_reduce(
                                out=va[:, :nt], in_=tmp[:, :nt, :],
                                axis=mybir.AxisListType.X, op=AoT.add)
                            vd = wp.tile([128, RC], dt.float32, tag="vd")
                            nc.vector.tensor_tensor(
                                out=tmp[:, :nt, :64 - dsh],
                                in0=gd[:, :nt, dsh:], in1=ohD[:, :nt, :64 - dsh],
                                op=AoT.mult)
                            if dsh:
                                nc.vector.memset(tmp[:, :nt, 64 - dsh:], 0.0)
                            nc.vector.tensor_reduce(
                                out=vd[:, :nt], in_=tmp[:, :nt, :],
                                axis=mybir.AxisListType.X, op=AoT.add)
                            ov = wp.tile([128, RC], dt.float32, tag="ov")
                            nc.vector.tensor_tensor(out=ov[:, :nt], in0=va[:, :nt],
                                                    in1=vd[:, :nt], op=AoT.add)
                            nc.sync.dma_start(out=oap[:, sl], in_=ov[:, :nt])
                    t0 += ntile

            heads(hia16_r1, haq_r1, hid16_r1, hdq_r1, runs1,
                  [(o_xlog, 0, 0)])
            heads(hia16_r2, haq_r2, hid16_r2, hdq_r2, runs2,
                  [(o_ytx, 0, 0), (o_yrx, 1, 1)])

    nc.compile()
    return nc


# -------------------------------------------------------------------- kernel
def kernel(x_ap, x_user, x_target, serves_src, serves_dst, senses_src,
           senses_dst, params):
    from concourse import bass_utils

    x_ap = np.asarray(x_ap, np.float32)
    x_user = np.asarray(x_user, np.float32)
    x_target = np.asarray(x_target, np.float32)
    serves_src = np.asarray(serves_src).astype(np.int64)
    serves_dst = np.asarray(serves_dst).astype(np.int64)
    senses_src = np.asarray(senses_src).astype(np.int64)
    senses_dst = np.asarray(senses_dst).astype(np.int64)

    meta, per_core = _prep_all(serves_src, serves_dst, senses_src, senses_dst)

    def g(*path):
        o = params
        for p in path:
            o = o[p]
        return np.asarray(o, np.float32)

    pwa = np.zeros((16, H), np.float32)
    pwa[:10] = g("proj", "ap", 0).T
    pwu = np.ascontiguousarray(g("proj", "user", 0).T)
    pwt = np.ascontiguousarray(g("proj", "target", 0).T)
    pb = np.tile(np.concatenate([g("proj", "ap", 1), g("proj", "user", 1),
                                 g("proj", "target", 1)])[None, :], (128, 1))
    rels = ["serves", "senses", "rev_serves", "rev_senses"]
    WLp = np.zeros((64, 8 * H), np.float32)
    WRp = np.zeros((64, 8 * H), np.float32)
    BLp = np.zeros((128, 8 * H), np.float32)
    for li, conv in enumerate(("conv1", "conv2")):
        for ri, r in enumerate(rels):
            col = (li * 4 + ri) * H
            WLp[:, col:col + H] = g(conv, r, 0).T
            BLp[:, col:col + H] = g(conv, r, 1)[None, :]
            WRp[:, col:col + H] = g(conv, r, 2).T
    WH = np.zeros((64, 8), np.float32)
    BH = np.zeros((128, 8), np.float32)
    WH[:, 0] = g("heads", "tau", 0)[0]
    BH[:, 0] = g("heads", "tau", 1)[0]
    WH[:, 1] = g("heads", "s", 0)[0]
    BH[:, 1] = g("heads", "s", 1)[0]
    WH[:, 2] = g("heads", "x", 0)[0, :H]
    WH[:, 3] = g("heads", "x", 0)[0, H:]
    BH[:, 2] = g("heads", "x", 1)[0]
    WH[:, 4] = g("heads", "ytx", 0)[0, :H]
    WH[:, 5] = g("heads", "ytx", 0)[0, H:]
    BH[:, 3] = g("heads", "ytx", 1)[0]
    WH[:, 6] = g("heads", "yrx", 0)[0, :H]
    WH[:, 7] = g("heads", "yrx", 0)[0, H:]
    BH[:, 4] = g("heads", "yrx", 1)[0]

    xapT = np.zeros((16, 512), np.float32)
    xapT[:10] = x_ap.T
    xu_pad = np.zeros((USH * NC, 8), np.float32)
    xu_pad[:N_USER] = x_user
    xt_pad = np.zeros((TSH * NC, 8), np.float32)
    xt_pad[:N_TARGET] = x_target

    in_maps = []
    for c in range(NC):
        io = per_core[c]
        in_maps.append({
            "xapT": xapT,
            "xuT_in": np.ascontiguousarray(xu_pad[USH * c:USH * (c + 1)].T),
            "xtT_in": np.ascontiguousarray(xt_pad[TSH * c:TSH * (c + 1)].T),
            "sa16_f1": io["sa16_f1"], "suoff_f1": io["suoff_f1"],
            "sa16_f2": io["sa16_f2"], "suoff_f2": io["suoff_f2"],
            "dq16_r1": io["dq16_r1"], "saoff_r1": io["saoff_r1"],
            "dq16_r2": io["dq16_r2"], "saoff_r2": io["saoff_r2"],
            "hia16_r1": io["hia16_r1"], "haq_r1": io["haq_r1"],
            "hid16_r1": io["hid16_r1"], "hdq_r1": io["hdq_r1"],
            "hia16_r2": io["hia16_r2"], "haq_r2": io["haq_r2"],
            "hid16_r2": io["hid16_r2"], "hdq_r2": io["hdq_r2"],
            "cnt_u": io["cnt_u"], "cnt_t": io["cnt_t"], "cnt_a": io["cnt_a"],
            "pwa": pwa, "pwu": pwu, "pwt": pwt, "pb": pb,
            "WL": WLp, "WR": WRp, "BL": BLp, "WHEAD": WH, "BHEAD": BH,
        })

    global _last_in_maps
    _last_in_maps = in_maps
    key = (meta["E1"], meta["E2"], meta["E1R"], meta["E2R"],
           tuple(meta["tpw_u"]), tuple(meta["tpw_t"]),
           tuple(meta["runs1"]), tuple(meta["runs2"]))
    if key not in _CACHE:
        _CACHE[key] = _build(meta)
    nc = _CACHE[key]

    res = bass_utils.run_bass_kernel_spmd(nc, in_maps,
                                          core_ids=list(range(NC)))

    tau = np.zeros(N_AP, np.float32)
    ot = res.results[0]["o_tau"]
    for s in range(4):
        tau[s * 128:(s + 1) * 128] = ot[:, s]
    s_out = np.zeros(N_TARGET, np.float32)
    x_log = np.zeros(E_SERVES, np.float32)
    ytx = np.zeros(E_SENSES, np.float32)
    yrx = np.zeros(E_SENSES, np.float32)
    for c in range(NC):
        r = res.results[c]
        sv = r["o_s"].T.reshape(-1)
        lo, hi = TSH * c, min(TSH * (c + 1), N_TARGET)
        if hi > lo:
            s_out[lo:hi] = sv[:hi - lo]
        for name, out_arr, perm in (("o_xlog", x_log, per_core[c]["perm_r1"]),
                                    ("o_ytx", ytx, per_core[c]["perm_r2"]),
                                    ("o_yrx", yrx, per_core[c]["perm_r2"])):
            vals = r[name].T.reshape(-1)
            mask = perm >= 0
            out_arr[perm[mask]] = vals[mask]
    return (tau, s_out, x_log, ytx, yrx)


# revision 11
# speedup vs baseline: 1.1418x; 1.1418x over previous
"""Trainium2 Bass kernel for the hetero-GNN message-passing model.

Distribution (8 NeuronCores, SPMD; full inputs in, full outputs out):
  - Destination-range sharding: each core owns 12544 users / 2560 targets;
    each edge is routed to the core owning its destination node and kept
    destination-sorted (CSR-style graph preprocessing on the host -- index
    plumbing only; every FLOP on tensor values runs on device).
  - Forward aggregation (AP -> user/target segment mean): SWDGE dma_gather
    fetches premultiplied AP-table rows per edge; a one-hot selector (DVE
    is_equal vs iota) scatters them into 128-destination PSUM windows via
    TensorE matmul accumulation; degree normalization + bias + dst-term +
    leaky-relu fused into the window drain.
  - Reverse aggregation (user/target -> 512 APs): per-core partials via
    gathered destination rows + one-hot PSUM matmuls into 4 AP slabs,
    AllReduce'd across cores.
  - Node tables cross cores via AllGather collectives; edge head outputs are
    computed in the per-core sorted order and un-permuted on the host.

Design note: this hardware build has no usable atomic scatter-add
(dma_scatter_add drops concurrent duplicate-row updates -- measured), so all
segment sums are sorted-order PSUM matmul accumulations.
"""

import numpy as np
import ml_dtypes

N_AP, N_USER, N_TARGET = 512, 100000, 20000
E_SERVES, E_SENSES = 2000000, 500000
H = 64
NEG_SLOPE = 0.01
NC = 8

USH = 12544           # users per core  (98 windows x 128)
TSH = 2560            # targets per core (20 windows x 128)
NW_U, NW_T = USH // 128, TSH // 128
GQ = 25088            # user-table quarter rows (int16-safe)
U_TBL = GQ * 4 + 128  # 100480
T_TBL = TSH * NC + 128
A_TBL = 640
RC = 16               # tiles per rev/head gather chunk (2048 edges)

_CACHE = {}
_last_in_maps = None


def _round_up(x, m):
    return (x + m - 1) // m * m


def _wrap16(idx):
    n = idx.shape[0]
    return np.ascontiguousarray(idx.astype(np.int16).reshape(n // 16, 16).T)


def _col128(arr, dtype):
    return np.ascontiguousarray(arr.reshape(-1, 128).T.astype(dtype))


# ------------------------------------------------------------------ host prep
def _prep_all(serves_src, serves_dst, senses_src, senses_dst):
    cores = []
    for c in range(NC):
        d = {}
        m1 = (serves_dst >= USH * c) & (serves_dst < USH * (c + 1))
        e1 = np.nonzero(m1)[0]
        src1, dst1 = serves_src[m1], serves_dst[m1]
        o = np.argsort(dst1, kind="stable")
        d["f1"] = (src1[o], (dst1 - USH * c)[o],
                   np.bincount((dst1 - USH * c) >> 7, minlength=NW_U))
        key = src1 >> 7
        o = np.argsort(key, kind="stable")
        d["r1"] = (src1[o], (dst1 - USH * c)[o], e1[o],
                   np.bincount(key[o], minlength=4))

        m2 = (senses_dst >= TSH * c) & (senses_dst < TSH * (c + 1))
        e2 = np.nonzero(m2)[0]
        src2, dst2 = senses_src[m2], senses_dst[m2]
        o = np.argsort(dst2, kind="stable")
        d["f2"] = (src2[o], (dst2 - TSH * c)[o],
                   np.bincount((dst2 - TSH * c) >> 7, minlength=NW_T))
        key = src2 >> 7
        o = np.argsort(key, kind="stable")
        d["r2"] = (src2[o], (dst2 - TSH * c)[o], e2[o],
                   np.bincount(key[o], minlength=4))
        cores.append(d)

    tpw_u = [int(_round_up(max(c["f1"][2][w] for c in cores), 128) // 128)
             for w in range(NW_U)]
    tpw_t = [int(_round_up(max(c["f2"][2][w] for c in cores), 128) // 128)
             for w in range(NW_T)]
    runs1 = [int(_round_up(max(c["r1"][3][k] for c in cores), 128) // 128)
             for k in range(4)]
    runs2 = [int(_round_up(max(c["r2"][3][k] for c in cores), 128) // 128)
             for k in range(4)]
    E1, E2 = sum(tpw_u) * 128, sum(tpw_t) * 128
    E1R, E2R = sum(runs1) * 128, sum(runs2) * 128
    meta = dict(tpw_u=tpw_u, tpw_t=tpw_t, runs1=runs1, runs2=runs2,
                E1=E1, E2=E2, E1R=E1R, E2R=E2R)

    deg_u = np.bincount(serves_dst, minlength=N_USER).astype(np.float32)
    deg_t = np.bincount(senses_dst, minlength=N_TARGET).astype(np.float32)
    deg_sa = np.bincount(serves_src, minlength=N_AP).astype(np.float32)
    deg_ta = np.bincount(senses_src, minlength=N_AP).astype(np.float32)
    cnt_a = np.zeros((128, 8), np.float32)
    for s in range(4):
        cnt_a[:, s] = deg_sa[s * 128:(s + 1) * 128]
        cnt_a[:, 4 + s] = deg_ta[s * 128:(s + 1) * 128]

    per_core = []
    for c in range(NC):
        io = {"cnt_a": cnt_a}
        for rel, tpw, E in (("f1", tpw_u, E1), ("f2", tpw_t, E2)):
            src_s, dst_s, counts = cores[c][rel]
            sa16 = np.full(E, 512, np.int64)
            suoff = np.zeros(E, np.float64)
            pos = ip = 0
            for w, tw in enumerate(tpw):
                n = int(counts[w])
                sa16[pos:pos + n] = src_s[ip:ip + n]
                suoff[pos:pos + n] = dst_s[ip:ip + n] & 127
                pos += tw * 128
                ip += n
            io[f"sa16_{rel}"] = _wrap16(sa16)
            io[f"suoff_{rel}"] = _col128(suoff, np.float16)
        for rel, runs, E in (("r1", runs1, E1R), ("r2", runs2, E2R)):
            src_o, dst_o, eglob, counts = cores[c][rel]
            dq16 = np.zeros(E, np.int64)
            saoff = np.full(E, 255.0, np.float64)
            perm = np.full(E, -1, np.int64)
            hia = np.full(E, 24, np.int64)
            haq = np.zeros(E, np.float64)
            hid = np.full(E, 196 if rel == "r1" else 80, np.int64)
            hdq = np.zeros(E, np.float64)
            pos = ip = 0
            for k, tw in enumerate(runs):
                n = int(counts[k])
                s_, d_ = src_o[ip:ip + n], dst_o[ip:ip + n]
                dq16[pos:pos + n] = d_
                saoff[pos:pos + n] = s_ & 127
                perm[pos:pos + n] = eglob[ip:ip + n]
                if rel == "r1":
                    hia[pos:pos + n] = 16 + (s_ >> 6)
                    haq[pos:pos + n] = s_ & 63
                    hid[pos:pos + n] = d_ >> 6
                    hdq[pos:pos + n] = d_ & 63
                else:
                    hia[pos:pos + n] = s_ >> 5
                    haq[pos:pos + n] = 2 * (s_ & 31)
                    hid[pos:pos + n] = d_ >> 5
                    hdq[pos:pos + n] = 2 * (d_ & 31)
                pos += tw * 128
                ip += n
            io[f"dq16_{rel}"] = _wrap16(dq16)
            io[f"saoff_{rel}"] = _col128(saoff, np.float16)
            io[f"hia16_{rel}"] = _wrap16(hia)
            io[f"haq_{rel}"] = _col128(haq, np.float16)
            io[f"hid16_{rel}"] = _wrap16(hid)
            io[f"hdq_{rel}"] = _col128(hdq, np.float16)
            io[f"perm_{rel}"] = perm
        du = np.zeros(USH, np.float32)
        hi = min(USH * (c + 1), N_USER)
        du[:hi - USH * c] = deg_u[USH * c:hi]
        io["cnt_u"] = _col128(du, np.float32)
        dt_ = np.zeros(TSH, np.float32)
        hi = min(TSH * (c + 1), N_TARGET)
        dt_[:hi - TSH * c] = deg_t[TSH * c:hi]
        io["cnt_t"] = _col128(dt_, np.float32)
        per_core.append(io)
    return meta, per_core


# ------------------------------------------------------------- device builder
def _build(meta):
    import concourse.bacc as bacc
    import concourse.mybir as mybir
    import concourse.tile as tile

    dt = mybir.dt
    AoT = mybir.AluOpType
    E1, E2, E1R, E2R = meta["E1"], meta["E2"], meta["E1R"], meta["E2R"]
    tpw_u, tpw_t = meta["tpw_u"], meta["tpw_t"]
    runs1, runs2 = meta["runs1"], meta["runs2"]
    TPW_MAX = max(max(tpw_u), max(tpw_t), RC)
    groups = [list(range(NC))]

    nc = bacc.Bacc("TRN2", target_bir_lowering=False, debug=False,
                   num_devices=NC)

    def din(name, shape, dtype):
        return nc.dram_tensor(name, shape, dtype, kind="ExternalInput").ap()

    def dout(name, shape, dtype):
        return nc.dram_tensor(name, shape, dtype, kind="ExternalOutput").ap()

    def dram(name, shape, dtype, shared=False):
        if shared:
            return nc.dram_tensor(name, shape, dtype, addr_space="Shared").ap()
        return nc.dram_tensor(name, shape, dtype).ap()

    xapT = din("xapT", [16, 512], dt.float32)
    xuT_in = din("xuT_in", [8, USH], dt.float32)
    xtT_in = din("xtT_in", [8, TSH], dt.float32)
    sa16_f1_in = din("sa16_f1", [16, E1 // 16], dt.int16)
    suoff_f1 = din("suoff_f1", [128, E1 // 128], dt.float16)
    sa16_f2_in = din("sa16_f2", [16, E2 // 16], dt.int16)
    suoff_f2 = din("suoff_f2", [128, E2 // 128], dt.float16)
    dq16_r1_in = din("dq16_r1", [16, E1R // 16], dt.int16)
    saoff_r1 = din("saoff_r1", [128, E1R // 128], dt.float16)
    dq16_r2_in = din("dq16_r2", [16, E2R // 16], dt.int16)
    saoff_r2 = din("saoff_r2", [128, E2R // 128], dt.float16)
    hia16_r1_in = din("hia16_r1", [16, E1R // 16], dt.int16)
    haq_r1 = din("haq_r1", [128, E1R // 128], dt.float16)
    hid16_r1_in = din("hid16_r1", [16, E1R // 16], dt.int16)
    hdq_r1 = din("hdq_r1", [128, E1R // 128], dt.float16)
    hia16_r2_in = din("hia16_r2", [16, E2R // 16], dt.int16)
    haq_r2 = din("haq_r2", [128, E2R // 128], dt.float16)
    hid16_r2_in = din("hid16_r2", [16, E2R // 16], dt.int16)
    hdq_r2 = din("hdq_r2", [128, E2R // 128], dt.float16)
    cnt_u = din("cnt_u", [128, NW_U], dt.float32)
    cnt_t = din("cnt_t", [128, NW_T], dt.float32)
    cnt_a = din("cnt_a", [128, 8], dt.float32)
    pwa = din("pwa", [16, H], dt.float32)
    pwu = din("pwu", [8, H], dt.float32)
    pwt = din("pwt", [8, H], dt.float32)
    pb = din("pb", [128, 3 * H], dt.float32)     # host-replicated biases
    WL = din("WL", [64, 8 * H], dt.float32)
    WR = din("WR", [64, 8 * H], dt.float32)
    BL = din("BL", [128, 8 * H], dt.float32)
    WHEAD = din("WHEAD", [64, 8], dt.float32)
    BHEAD = din("BHEAD", [128, 8], dt.float32)

    o_tau = dout("o_tau", [128, 4], dt.float32)
    o_s = dout("o_s", [128, NW_T], dt.float32)
    o_xlog = dout("o_xlog", [128, E1R // 128], dt.float32)
    o_ytx = dout("o_ytx", [128, E2R // 128], dt.float32)
    o_yrx = dout("o_yrx", [128, E2R // 128], dt.float32)

    sa16_f1 = dram("sa16_f1_d", [128, E1 // 16], dt.int16)
    sa16_f2 = dram("sa16_f2_d", [128, E2 // 16], dt.int16)
    dq16_r1 = dram("dq16_r1_d", [128, E1R // 16], dt.int16)
    dq16_r2 = dram("dq16_r2_d", [128, E2R // 16], dt.int16)
    hia16_r1 = dram("hia16_r1_d", [128, E1R // 16], dt.int16)
    hid16_r1 = dram("hid16_r1_d", [128, E1R // 16], dt.int16)
    hia16_r2 = dram("hia16_r2_d", [128, E2R // 16], dt.int16)
    hid16_r2 = dram("hid16_r2_d", [128, E2R // 16], dt.int16)
    tblA = dram("tblA", [A_TBL, 64], dt.float32)
    tblU = dram("tblU", [USH + 128, 64], dt.float32)
    tblT = dram("tblT", [TSH + 128, 64], dt.float32)
    rev_in = dram("rev_in", [8, 128, H], dt.float32)
    rev_out = dram("rev_out", [8, 128, H], dt.float32, shared=True)
    tblHA = dram("tblHA", [32, 64], dt.float32)
    tblHU = dram("tblHU", [200, 64], dt.float32)
    tblHT = dram("tblHT", [84, 64], dt.float32)

    with tile.TileContext(nc) as tc:
        with (tc.tile_pool(name="c", bufs=1) as cp,
              tc.tile_pool(name="w", bufs=2) as wp,
              tc.tile_pool(name="g", bufs=2) as gp,
              tc.tile_pool(name="p", bufs=6, space="PSUM") as pp,
              tc.tile_pool(name="pt", bufs=2, space="PSUM") as ptp):

            # ------- replicate wrapped idx arrays to all partition groups -------
            for _src, _dst in ((sa16_f1_in, sa16_f1), (sa16_f2_in, sa16_f2),
                               (dq16_r1_in, dq16_r1), (dq16_r2_in, dq16_r2),
                               (hia16_r1_in, hia16_r1), (hid16_r1_in, hid16_r1),
                               (hia16_r2_in, hia16_r2), (hid16_r2_in, hid16_r2)):
                for _k in range(8):
                    nc.sync.dma_start(out=_dst[16 * _k:16 * (_k + 1), :],
                                      in_=_src[:])

            # ---------------- constants ----------------
            iota_i = cp.tile([128, 128], dt.int16, tag="iota_i")
            nc.gpsimd.iota(iota_i[:], [[1, 128]], channel_multiplier=0)
            iota_f = cp.tile([128, 128], dt.float16, tag="iota_f")
            nc.vector.tensor_copy(iota_f[:], iota_i[:])
            iota_pi = cp.tile([128, 1], dt.int16, tag="iota_pi")
            nc.gpsimd.iota(iota_pi[:], [[0, 1]], channel_multiplier=1)
            iota_pf = cp.tile([128, 1], dt.float16, tag="iota_pf")
            nc.vector.tensor_copy(iota_pf[:], iota_pi[:])
            ident = cp.tile([128, 128], dt.float32, tag="ident")
            nc.vector.tensor_tensor(out=ident[:],
                                    in0=iota_pf[:].to_broadcast([128, 128]),
                                    in1=iota_f[:], op=AoT.is_equal)

            WLs = cp.tile([64, 8 * H], dt.float32, tag="WLs")
            nc.sync.dma_start(out=WLs[:], in_=WL[:])
            WRs = cp.tile([64, 8 * H], dt.float32, tag="WRs")
            nc.sync.dma_start(out=WRs[:], in_=WR[:])
            WRb = cp.tile([64, 8 * H], dt.bfloat16, tag="WRb")
            nc.vector.tensor_copy(WRb[:], WRs[:])
            BLs = cp.tile([128, 8 * H], dt.float32, tag="BLs")
            nc.sync.dma_start(out=BLs[:], in_=BL[:])
            WHs = cp.tile([64, 8], dt.float32, tag="WHs")
            nc.sync.dma_start(out=WHs[:], in_=WHEAD[:])
            WHb = cp.tile([64, 8], dt.bfloat16, tag="WHb")
            nc.vector.tensor_copy(WHb[:], WHs[:])
            BHs = cp.tile([128, 8], dt.float32, tag="BHs")
            nc.sync.dma_start(out=BHs[:], in_=BHEAD[:])

            def load_inv(ap, n, tag):
                t = cp.tile([128, n], dt.float32, tag=tag)
                nc.sync.dma_start(out=t[:], in_=ap[:])
                nc.vector.tensor_scalar_max(t[:], t[:], 1.0)
                nc.vector.reciprocal(t[:], t[:])
                return t
            inv_u = load_inv(cnt_u, NW_U, "inv_u")
            inv_t = load_inv(cnt_t, NW_T, "inv_t")
            inv_a = load_inv(cnt_a, 8, "inv_a")

            xaT = cp.tile([64, 512], dt.float32, tag="xaT")
            xuT = cp.tile([64, USH], dt.bfloat16, tag="xuT")
            xtT = cp.tile([64, TSH], dt.bfloat16, tag="xtT")
            xuT2 = cp.tile([64, USH], dt.bfloat16, tag="xuT2")
            xtT2 = cp.tile([64, TSH], dt.bfloat16, tag="xtT2")

            def lrelu(ap, shape):
                t = wp.tile(shape, dt.float32, tag="lr")
                nc.vector.tensor_scalar_mul(t[:], ap, NEG_SLOPE)
                nc.vector.tensor_tensor(out=ap, in0=ap, in1=t[:], op=AoT.max)

            def transpose_to(destT_slice, ob_ap, out_dtype_bf):
                """destT_slice [64, 128] <- ob [128, 64]^T via PE."""
                psT = ptp.tile([64, 128], dt.float32, tag="pt")
                nc.tensor.transpose(out=psT[:], in_=ob_ap, identity=ident[:])
                nc.vector.tensor_copy(destT_slice, psT[:])

            # ---------------- projections ----------------
            xapT_s = wp.tile([16, 512], dt.float32, tag="xapT_s")
            nc.sync.dma_start(out=xapT_s[:], in_=xapT[:])
            pwa_s = cp.tile([16, H], dt.float32, tag="pwa_s")
            nc.sync.dma_start(out=pwa_s[:], in_=pwa[:])
            pb_s = cp.tile([128, 3 * H], dt.float32, tag="pb_s")
            nc.sync.dma_start(out=pb_s[:], in_=pb[:])
            for s in range(4):
                ps = pp.tile([128, H], dt.float32, tag="p")
                nc.tensor.matmul(ps[:], lhsT=xapT_s[:, s * 128:(s + 1) * 128],
                                 rhs=pwa_s[:], start=True, stop=True)
                ob = wp.tile([128, H], dt.float32, tag="ob")
                nc.vector.scalar_tensor_tensor(
                    out=ob[:], in0=ps[:], scalar=1.0, in1=pb_s[:, 0:H],
                    op0=AoT.mult, op1=AoT.add)
                transpose_to(xaT[:, s * 128:(s + 1) * 128], ob[:], False)

            def proj_nodes(xT_in, pw_ap, bias_off, destT, nwin, tbl_loc, ncols):
                pw_s = wp.tile([8, H], dt.float32, tag="pw_s")
                nc.sync.dma_start(out=pw_s[:], in_=pw_ap[:])
                for w in range(nwin):
                    xTw = wp.tile([8, 128], dt.float32, tag="xTw")
                    nc.sync.dma_start(out=xTw[:], in_=xT_in[:, w * 128:(w + 1) * 128])
                    ps = pp.tile([128, H], dt.float32, tag="p")
                    nc.tensor.matmul(ps[:], lhsT=xTw[:],
                                     rhs=pw_s[:], start=True, stop=True)
                    ob = wp.tile([128, H], dt.float32, tag="ob")
                    nc.vector.scalar_tensor_tensor(
                        out=ob[:], in0=ps[:], scalar=1.0,
                        in1=pb_s[:, bias_off:bias_off + H],
                        op0=AoT.mult, op1=AoT.add)
                    transpose_to(destT[:, w * 128:(w + 1) * 128], ob[:], True)
                    nc.sync.dma_start(out=tbl_loc[w * 128:(w + 1) * 128, :],
                                      in_=ob[:])

            proj_nodes(xuT_in, pwu, H, xuT, NW_U, tblU, USH)
            proj_nodes(xtT_in, pwt, 2 * H, xtT, NW_T, tblT, TSH)

            def build_tblA(wl_col):
                for s in range(4):
                    ps = pp.tile([128, H], dt.float32, tag="p")
                    nc.tensor.matmul(ps[:], lhsT=xaT[:, s * 128:(s + 1) * 128],
                                     rhs=WLs[:, wl_col:wl_col + H],
                                     start=True, stop=True)
                    pk = wp.tile([128, H], dt.float32, tag="pk")
                    nc.vector.tensor_copy(pk[:], ps[:])
                    nc.sync.dma_start(out=tblA[s * 128:(s + 1) * 128, :],
                                      in_=pk[:])
                zr = wp.tile([128, H], dt.float32, tag="pk")
                nc.vector.memset(zr[:], 0.0)
                nc.sync.dma_start(out=tblA[512:640, :], in_=zr[:])

            # ------------- forward pass -------------
            def fwd(sa16, suoff, tpw, inv_dst, col, srcT, destT, tbl_loc, layer):
                t0 = 0
                for w, ntile in enumerate(tpw):
                    psd = pp.tile([128, H], dt.float32, tag="p")
                    nc.tensor.matmul(psd[:], lhsT=srcT[:, w * 128:(w + 1) * 128],
                                     rhs=WRb[:, col:col + H], start=True, stop=True)
                    ob = wp.tile([128, H], dt.float32, tag="ob")
                    if ntile > 0:
                        ix = gp.tile([128, TPW_MAX * 8], dt.int16, tag="ix")
                        nc.sync.dma_start(out=ix[:, :ntile * 8],
                                          in_=sa16[:, t0 * 8:(t0 + ntile) * 8])
                        g = gp.tile([128, TPW_MAX, 64], dt.float32, tag="gg")
                        nc.gpsimd.dma_gather(
                            g[:, :ntile, :], tblA[:], ix[:, :ntile * 8],
                            ntile * 128, ntile * 128, 64, single_packet=False)
                        so = gp.tile([128, TPW_MAX], dt.float16, tag="so")
                        nc.sync.dma_start(out=so[:, :ntile],
                                          in_=suoff[:, t0:t0 + ntile])
                        oh = gp.tile([128, TPW_MAX, 128], dt.float32, tag="oh")
                        nc.vector.tensor_tensor(
                            out=oh[:, :ntile, :],
                            in0=so[:, :ntile, None].to_broadcast([128, ntile, 128]),
                            in1=iota_f[:, None, :].to_broadcast([128, ntile, 128]),
                            op=AoT.is_equal)
                        psa = pp.tile([128, H], dt.float32, tag="p")
                        for t in range(ntile):
                            nc.tensor.matmul(psa[:], lhsT=oh[:, t, :],
                                             rhs=g[:, t, :],
                                             start=(t == 0), stop=(t == ntile - 1))
                        nc.vector.tensor_scalar(
                            out=ob[:], in0=psa[:], scalar1=inv_dst[:, w:w + 1],
                            scalar2=None, op0=AoT.mult)
                        nc.vector.tensor_tensor(out=ob[:], in0=ob[:], in1=psd[:],
                                                op=AoT.add)
                    else:
                        nc.vector.tensor_copy(ob[:], psd[:])
                    nc.vector.tensor_tensor(out=ob[:], in0=ob[:],
                                            in1=BLs[:, col:col + H], op=AoT.add)
                    lrelu(ob[:], [128, H])
                    transpose_to(destT[:, w * 128:(w + 1) * 128], ob[:], True)
                    if layer == 1:
                        nc.sync.dma_start(out=tbl_loc[w * 128:(w + 1) * 128, :],
                                          in_=ob[:])
                    t0 += ntile

            # ------------- reverse pass -------------
            def rev(dq16, saoff, runs, tbl, revacc, acc_off):
                t0 = 0
                for s, ntile in enumerate(runs):
                    if ntile == 0:
                        continue
                    psr = pp.tile([128, H], dt.float32, tag="p")
                    nchunks = (ntile + RC - 1) // RC
                    for ci in range(nchunks):
                        c0 = ci * RC
                        nt = min(RC, ntile - c0)
                        ix = gp.tile([128, TPW_MAX * 8], dt.int16, tag="ix")
                        nc.sync.dma_start(
                            out=ix[:, :nt * 8],
                            in_=dq16[:, (t0 + c0) * 8:(t0 + c0 + nt) * 8])
                        g = gp.tile([128, TPW_MAX, 64], dt.float32, tag="gg")
                        nc.gpsimd.dma_gather(
                            g[:, :nt, :], tbl[:], ix[:, :nt * 8],
                            nt * 128, nt * 128, 64, single_packet=False)
                        so = gp.tile([128, TPW_MAX], dt.float16, tag="so")
                        nc.sync.dma_start(out=so[:, :nt],
                                          in_=saoff[:, t0 + c0:t0 + c0 + nt])
                        oh = gp.tile([128, TPW_MAX, 128], dt.float32, tag="oh")
                        nc.vector.tensor_tensor(
                            out=oh[:, :nt, :],
                            in0=so[:, :nt, None].to_broadcast([128, nt, 128]),
                            in1=iota_f[:, None, :].to_broadcast([128, nt, 128]),
                            op=AoT.is_equal)
                        for t in range(nt):
                            nc.tensor.matmul(
                                psr[:], lhsT=oh[:, t, :], rhs=g[:, t, :],
                                start=(ci == 0 and t == 0),
                                stop=(ci == nchunks - 1 and t == nt - 1))
                    nc.vector.tensor_tensor(
                        out=revacc[:, acc_off + s, :],
                        in0=revacc[:, acc_off + s, :], in1=psr[:], op=AoT.add)
                    t0 += ntile

            # ------------- AP update -------------
            def ap_update(revacc, layer):
                nc.sync.dma_start(out=rev_in[:], in_=revacc[:])
                nc.gpsimd.collective_compute(
                    "AllReduce", AoT.add, replica_groups=groups,
                    ins=[rev_in[:]], outs=[rev_out[:]])
                ra = wp.tile([128, 8, H], dt.float32, tag="ra")
                nc.sync.dma_start(out=ra[:], in_=rev_out[:])
                base = (layer - 1) * 4 * H
                wrsum = wp.tile([64, H], dt.float32, tag="wrsum")
                nc.vector.tensor_tensor(out=wrsum[:],
                                        in0=WRs[:, base + 2 * H:base + 3 * H],
                                        in1=WRs[:, base + 3 * H:base + 4 * H],
                                        op=AoT.add)
                for s in range(4):
                    ms = wp.tile([128, H], dt.float32, tag="ms")
                    nc.vector.tensor_scalar(
                        out=ms[:], in0=ra[:, s, :], scalar1=inv_a[:, s:s + 1],
                        scalar2=None, op0=AoT.mult)
                    msT = ptp.tile([64, 128], dt.float32, tag="pt")
                    nc.tensor.transpose(out=msT[:], in_=ms[:],
                                        identity=ident[:])
                    msTs = wp.tile([64, 128], dt.float32, tag="msTs")
                    nc.vector.tensor_copy(msTs[:], msT[:])
                    mt = wp.tile([128, H], dt.float32, tag="mt")
                    nc.vector.tensor_scalar(
                        out=mt[:], in0=ra[:, 4 + s, :],
                        scalar1=inv_a[:, 4 + s:5 + s], scalar2=None, op0=AoT.mult)
                    mtT = ptp.tile([64, 128], dt.float32, tag="pt")
                    nc.tensor.transpose(out=mtT[:], in_=mt[:],
                                        identity=ident[:])
                    mtTs = wp.tile([64, 128], dt.float32, tag="mtTs")
                    nc.vector.tensor_copy(mtTs[:], mtT[:])
                    ps = pp.tile([128, H], dt.float32, tag="p")
                    nc.tensor.matmul(ps[:], lhsT=msTs[:],
                                     rhs=WLs[:, base + 2 * H:base + 3 * H],
                                     start=True, stop=False)
                    nc.tensor.matmul(ps[:], lhsT=mtTs[:],
                                     rhs=WLs[:, base + 3 * H:base + 4 * H],
                                     start=False, stop=False)
                    nc.tensor.matmul(ps[:], lhsT=xaT[:, s * 128:(s + 1) * 128],
                                     rhs=wrsum[:], start=False, stop=True)
                    ob = wp.tile([128, H], dt.float32, tag="ob")
                    nc.vector.tensor_tensor(out=ob[:], in0=ps[:],
                                            in1=BLs[:, base + 2 * H:base + 3 * H],
                                            op=AoT.add)
                    nc.vector.tensor_tensor(out=ob[:], in0=ob[:],
                                            in1=BLs[:, base + 3 * H:base + 4 * H],
                                            op=AoT.add)
                    lrelu(ob[:], [128, H])
                    # xaT is read by matmuls above for all slabs; stage new
                    # values and write back after? (slab s update only reads
                    # slab s of xaT -> safe in-place per slab)
                    transpose_to(xaT[:, s * 128:(s + 1) * 128], ob[:], False)

            # =================== layer 1 ===================
            revacc = cp.tile([128, 8, H], dt.float32, tag="revacc")
            nc.vector.memset(revacc[:], 0.0)
            rev(dq16_r1, saoff_r1, runs1, tblU, revacc, 0)
            rev(dq16_r2, saoff_r2, runs2, tblT, revacc, 4)
            build_tblA(0 * H)
            fwd(sa16_f1, suoff_f1, tpw_u, inv_u, 0 * H, xuT, xuT2, tblU, 1)
            build_tblA(1 * H)
            fwd(sa16_f2, suoff_f2, tpw_t, inv_t, 1 * H, xtT, xtT2, tblT, 1)
            ap_update(revacc, 1)

            # =================== layer 2 ===================
            revacc2 = cp.tile([128, 8, H], dt.float32, tag="revacc2")
            nc.vector.memset(revacc2[:], 0.0)
            rev(dq16_r1, saoff_r1, runs1, tblU, revacc2, 0)
            rev(dq16_r2, saoff_r2, runs2, tblT, revacc2, 4)
            build_tblA(4 * H)
            fwd(sa16_f1, suoff_f1, tpw_u, inv_u, 4 * H, xuT2, xuT, None, 2)
            build_tblA(5 * H)
            fwd(sa16_f2, suoff_f2, tpw_t, inv_t, 5 * H, xtT2, xtT, None, 2)
            ap_update(revacc2, 2)

            # =================== head node-scalars ===================
            taus = wp.tile([128, 4], dt.float32, tag="taus")
            for s in range(4):
                ps = pp.tile([128, H], dt.float32, tag="p")
                nc.tensor.matmul(ps[:, :8], lhsT=xaT[:, s * 128:(s + 1) * 128],
                                 rhs=WHs[:], start=True, stop=True)
                hb = wp.tile([128, 1], dt.float32, tag="hb")
                nc.vector.tensor_tensor(out=hb[:], in0=ps[:, 0:1],
                                        in1=BHs[:, 0:1], op=AoT.add)
                nc.vector.tensor_copy(taus[:, s:s + 1], hb[:, 0:1])
                # pa (no bias; bx folded into pu) -> tblHA rows 16+2s..17+2s
                pa2 = wp.tile([128, 1], dt.float32, tag="pa2")
                nc.vector.tensor_copy(pa2[:], ps[:, 2:3])
                nc.sync.dma_start(out=tblHA[16 + 2 * s:18 + 2 * s, :],
                                  in_=pa2[:])
                # (qa, ra) interleaved, unbiased -> tblHA rows 4s..4s+3
                hb2 = wp.tile([128, 2], dt.float32, tag="hb2")
                nc.vector.tensor_copy(hb2[:, 0:1], ps[:, 4:5])
                nc.vector.tensor_copy(hb2[:, 1:2], ps[:, 6:7])
                nc.sync.dma_start(out=tblHA[4 * s:4 * s + 4, :], in_=hb2[:])
            nc.sync.dma_start(out=o_tau[:], in_=taus[:])
            zr8 = wp.tile([128, 2], dt.float32, tag="zr8")
            nc.vector.memset(zr8[:], 0.0)
            nc.sync.dma_start(out=tblHA[24:28, :], in_=zr8[:])
            nc.sync.dma_start(out=tblHA[28:32, :], in_=zr8[:])
            nc.sync.dma_start(out=tblHU[196:200, :], in_=zr8[:])
            nc.sync.dma_start(out=tblHT[80:84, :], in_=zr8[:])

            for w in range(NW_U):
                ps = pp.tile([128, H], dt.float32, tag="p")
                nc.tensor.matmul(ps[:, :8], lhsT=xuT[:, w * 128:(w + 1) * 128],
                                 rhs=WHb[:], start=True, stop=True)
                pu = wp.tile([128, 1], dt.float32, tag="pu")
                nc.vector.tensor_tensor(out=pu[:], in0=ps[:, 3:4],
                                        in1=BHs[:, 2:3], op=AoT.add)
                nc.sync.dma_start(out=tblHU[2 * w:2 * w + 2, :], in_=pu[:])
            souts = wp.tile([128, NW_T], dt.float32, tag="souts")
            for w in range(NW_T):
                ps = pp.tile([128, H], dt.float32, tag="p")
                nc.tensor.matmul(ps[:, :8], lhsT=xtT[:, w * 128:(w + 1) * 128],
                                 rhs=WHb[:], start=True, stop=True)
                nc.vector.tensor_tensor(out=souts[:, w:w + 1], in0=ps[:, 1:2],
                                        in1=BHs[:, 1:2], op=AoT.add)
                qts = wp.tile([128, 2], dt.float32, tag="qts")
                nc.vector.tensor_tensor(out=qts[:, 0:1], in0=ps[:, 5:6],
                                        in1=BHs[:, 3:4], op=AoT.add)
                nc.vector.tensor_tensor(out=qts[:, 1:2], in0=ps[:, 7:8],
                                        in1=BHs[:, 4:5], op=AoT.add)
                nc.sync.dma_start(out=tblHT[4 * w:4 * w + 4, :], in_=qts[:])
            nc.sync.dma_start(out=o_s[:], in_=souts[:])

            # =================== per-edge heads ===================
            def heads(hia16, haq, hid16, hdq, runs, outs_spec):
                """outs_spec: list of (out_ap, a_shift, d_shift)."""
                t0 = 0
                for k, ntile in enumerate(runs):
                    if ntile == 0:
                        continue
                    for c0 in range(0, ntile, RC):
                        nt = min(RC, ntile - c0)
                        sl16 = slice((t0 + c0) * 8, (t0 + c0 + nt) * 8)
                        sl = slice(t0 + c0, t0 + c0 + nt)
                        ixa = gp.tile([128, TPW_MAX * 8], dt.int16, tag="ix")
                        nc.sync.dma_start(out=ixa[:, :nt * 8], in_=hia16[:, sl16])
                        ga = gp.tile([128, RC, 64], dt.float32, tag="hga")
                        nc.gpsimd.dma_gather(
                            ga[:, :nt, :], tblHA[:], ixa[:, :nt * 8],
                            nt * 128, nt * 128, 64, single_packet=False)
                        ixd = gp.tile([128, TPW_MAX * 8], dt.int16, tag="ix2")
                        nc.sync.dma_start(out=ixd[:, :nt * 8], in_=hid16[:, sl16])
                        gd = gp.tile([128, RC, 64], dt.float32, tag="hgd")
                        nc.gpsimd.dma_gather(
                            gd[:, :nt, :], tblHU[:] if runs is runs1 else tblHT[:],
                            ixd[:, :nt * 8],
                            nt * 128, nt * 128, 64, single_packet=False)
                        qa_ = gp.tile([128, TPW_MAX], dt.float16, tag="so")
                        nc.sync.dma_start(out=qa_[:, :nt], in_=haq[:, sl])
                        qd_ = gp.tile([128, TPW_MAX], dt.float16, tag="so2")
                        nc.sync.dma_start(out=qd_[:, :nt], in_=hdq[:, sl])
                        ohA = gp.tile([128, RC, 64], dt.float32, tag="ohA")
                        nc.vector.tensor_tensor(
                            out=ohA[:, :nt, :],
                            in0=qa_[:, :nt, None].to_broadcast([128, nt, 64]),
                            in1=iota_f[:, None, :64].to_broadcast([128, nt, 64]),
                            op=AoT.is_equal)
                        ohD = gp.tile([128, RC, 64], dt.float32, tag="ohD")
                        nc.vector.tensor_tensor(
                            out=ohD[:, :nt, :],
                            in0=qd_[:, :nt, None].to_broadcast([128, nt, 64]),
                            in1=iota_f[:, None, :64].to_broadcast([128, nt, 64]),
                            op=AoT.is_equal)
                        tmp = gp.tile([128, RC, 64], dt.float32, tag="tmp")
                        for oap, ash, dsh in outs_spec:
                            va = wp.tile([128, RC], dt.float32, tag="va")
                            nc.vector.tensor_tensor(
                                out=tmp[:, :nt, :64 - ash],
                                in0=ga[:, :nt, ash:], in1=ohA[:, :nt, :64 - ash],
                                op=AoT.mult)
                            if ash:
                                nc.vector.memset(tmp[:, :nt, 64 - ash:], 0.0)
                            nc.vector.tensor_reduce(
                                out=va[:, :nt], in_=tmp[:, :nt, :],
                                axis=mybir.AxisListType.X, op=AoT.add)
                            vd = wp.tile([128, RC], dt.float32, tag="vd")
                            nc.vector.tensor_tensor(
                                out=tmp[:, :nt, :64 - dsh],
                                in0=gd[:, :nt, dsh:], in1=ohD[:, :nt, :64 - dsh],
                                op=AoT.mult)
                            if dsh:
                                nc.vector.memset(tmp[:, :nt, 64 - dsh:], 0.0)
                            nc.vector.tensor_reduce(
                                out=vd[:, :nt], in_=tmp[:, :nt, :],
                                axis=mybir.AxisListType.X, op=AoT.add)
                            ov = wp.tile([128, RC], dt.float32, tag="ov")
                            nc.vector.tensor_tensor(out=ov[:, :nt], in0=va[:, :nt],
                                                    in1=vd[:, :nt], op=AoT.add)
                            nc.sync.dma_start(out=oap[:, sl], in_=ov[:, :nt])
                    t0 += ntile

            heads(hia16_r1, haq_r1, hid16_r1, hdq_r1, runs1,
                  [(o_xlog, 0, 0)])
            heads(hia16_r2, haq_r2, hid16_r2, hdq_r2, runs2,
                  [(o_ytx, 0, 0), (o_yrx, 1, 1)])

    nc.compile()
    return nc


# -------------------------------------------------------------------- kernel
def kernel(x_ap, x_user, x_target, serves_src, serves_dst, senses_src,
           senses_dst, params):
    from concourse import bass_utils

    x_ap = np.asarray(x_ap, np.float32)
    x_user = np.asarray(x_user, np.float32)
    x_target = np.asarray(x_target, np.float32)
    serves_src = np.asarray(serves_src).astype(np.int64)
    serves_dst = np.asarray(serves_dst).astype(np.int64)
    senses_src = np.asarray(senses_src).astype(np.int64)
    senses_dst = np.asarray(senses_dst).astype(np.int64)

    meta, per_core = _prep_all(serves_src, serves_dst, senses_src, senses_dst)

    def g(*path):
        o = params
        for p in path:
            o = o[p]
        return np.asarray(o, np.float32)

    pwa = np.zeros((16, H), np.float32)
    pwa[:10] = g("proj", "ap", 0).T
    pwu = np.ascontiguousarray(g("proj", "user", 0).T)
    pwt = np.ascontiguousarray(g("proj", "target", 0).T)
    pb = np.tile(np.concatenate([g("proj", "ap", 1), g("proj", "user", 1),
                                 g("proj", "target", 1)])[None, :], (128, 1))
    rels = ["serves", "senses", "rev_serves", "rev_senses"]
    WLp = np.zeros((64, 8 * H), np.float32)
    WRp = np.zeros((64, 8 * H), np.float32)
    BLp = np.zeros((128, 8 * H), np.float32)
    for li, conv in enumerate(("conv1", "conv2")):
        for ri, r in enumerate(rels):
            col = (li * 4 + ri) * H
            WLp[:, col:col + H] = g(conv, r, 0).T
            BLp[:, col:col + H] = g(conv, r, 1)[None, :]
            WRp[:, col:col + H] = g(conv, r, 2).T
    WH = np.zeros((64, 8), np.float32)
    BH = np.zeros((128, 8), np.float32)
    WH[:, 0] = g("heads", "tau", 0)[0]
    BH[:, 0] = g("heads", "tau", 1)[0]
    WH[:, 1] = g("heads", "s", 0)[0]
    BH[:, 1] = g("heads", "s", 1)[0]
    WH[:, 2] = g("heads", "x", 0)[0, :H]
    WH[:, 3] = g("heads", "x", 0)[0, H:]
    BH[:, 2] = g("heads", "x", 1)[0]
    WH[:, 4] = g("heads", "ytx", 0)[0, :H]
    WH[:, 5] = g("heads", "ytx", 0)[0, H:]
    BH[:, 3] = g("heads", "ytx", 1)[0]
    WH[:, 6] = g("heads", "yrx", 0)[0, :H]
    WH[:, 7] = g("heads", "yrx", 0)[0, H:]
    BH[:, 4] = g("heads", "yrx", 1)[0]

    xapT = np.zeros((16, 512), np.float32)
    xapT[:10] = x_ap.T
    xu_pad = np.zeros((USH * NC, 8), np.float32)
    xu_pad[:N_USER] = x_user
    xt_pad = np.zeros((TSH * NC, 8), np.float32)
    xt_pad[:N_TARGET] = x_target

    in_maps = []
    for c in range(NC):
        io = per_core[c]
        in_maps.append({
            "xapT": xapT,
            "xuT_in": np.ascontiguousarray(xu_pad[USH * c:USH * (c + 1)].T),
            "xtT_in": np.ascontiguousarray(xt_pad[TSH * c:TSH * (c + 1)].T),
            "sa16_f1": io["sa16_f1"], "suoff_f1": io["suoff_f1"],
            "sa16_f2": io["sa16_f2"], "suoff_f2": io["suoff_f2"],
            "dq16_r1": io["dq16_r1"], "saoff_r1": io["saoff_r1"],
            "dq16_r2": io["dq16_r2"], "saoff_r2": io["saoff_r2"],
            "hia16_r1": io["hia16_r1"], "haq_r1": io["haq_r1"],
            "hid16_r1": io["hid16_r1"], "hdq_r1": io["hdq_r1"],
            "hia16_r2": io["hia16_r2"], "haq_r2": io["haq_r2"],
            "hid16_r2": io["hid16_r2"], "hdq_r2": io["hdq_r2"],
            "cnt_u": io["cnt_u"], "cnt_t": io["cnt_t"], "cnt_a": io["cnt_a"],
            "pwa": pwa, "pwu": pwu, "pwt": pwt, "pb": pb,
            "WL": WLp, "WR": WRp, "BL": BLp, "WHEAD": WH, "BHEAD": BH,
        })

    global _last_in_maps
    _last_in_maps = in_maps
    key = (meta["E1"], meta["E2"], meta["E1R"], meta["E2R"],
           tuple(meta["tpw_u"]), tuple(meta["tpw_t"]),
           tuple(meta["runs1"]), tuple(meta["runs2"]))
    if key not in _CACHE:
        _CACHE[key] = _build(meta)
    nc = _CACHE[key]

    res = bass_utils.run_bass_kernel_spmd(nc, in_maps,
                                          core_ids=list(range(NC)))

    tau = np.zeros(N_AP, np.float32)
    ot = res.results[0]["o_tau"]
    for s in range(4):
        tau[s * 128:(s + 1) * 128] = ot[:, s]
    s_out = np.zeros(N_TARGET, np.float32)
    x_log = np.zeros(E_SERVES, np.float32)
    ytx = np.zeros(E_SENSES, np.float32)
    yrx = np.zeros(E_SENSES, np.float32)
    for c in range(NC):
        r = res.results[c]
        sv = r["o_s"].T.reshape(-1)
        lo, hi = TSH * c, min(TSH * (c + 1), N_TARGET)
        if hi > lo:
            s_out[lo:hi] = sv[:hi - lo]
        for name, out_arr, perm in (("o_xlog", x_log, per_core[c]["perm_r1"]),
                                    ("o_ytx", ytx, per_core[c]["perm_r2"]),
                                    ("o_yrx", yrx, per_core[c]["perm_r2"])):
            vals = r[name].T.reshape(-1)
            mask = perm >= 0
            out_arr[perm[mask]] = vals[mask]
    return (tau, s_out, x_log, ytx, yrx)
